# revision 1
# baseline (speedup 1.0000x reference)
"""Trainium2 Bass kernel for multi-head causal attention with RoPE.

Model (per reference):
  B=2, S=2048, D=4096, H=32 heads, HD=128.
  out = softmax(rope(x@wq) @ rope(x@wk)^T / sqrt(HD) + mask) @ (x@wv) @ wo

Sharding: tensor-parallel over heads. Core c in 0..7 owns heads 4c..4c+3:
wq/wk/wv column-sharded, wo row-sharded; each core produces a full-shape
partial output and the host sums the 8 partials (the all-reduce).

Per-core pipeline (all matmuls in fp32r — bit-identical to fp32 on this
HW, 4x faster at N>=512):
  Phase 1a: Q,K projections emitted directly in transposed [HD, tok]
            layout (lhsT = weight tile, rhs = xT tile), RoPE applied in
            rotate-half form (host permutes wq/wk columns evens-then-odds
            per head so the pair-swap becomes a partition-block swap done
            with two SBUF->SBUF DMAs), spilled to DRAM scratch.
  Phase 1b: V projection in natural [tok, d] layout, spilled to DRAM.
  Phase 2:  per (head,batch): S^T[k,q] = K^T-tile' @ Q^T chunk computed
            256 q-columns at a time; causal mask added only on diagonal
            128x256 blocks, k-tiles above the diagonal skipped; exp on
            ScalarE (no max subtraction - scores are bounded, softmax is
            shift-invariant so this matches the reference up to
            rounding); column sums via ones-vector matmul; PV
            accumulated in PSUM as ho^T[d,q]; normalization applied
            after PV via a gpsimd partition-broadcast reciprocal.
  Phase 3:  out_partial[tok,:] += ho^T_h' @ wo_rows_h accumulated over
            the 4 local heads in PSUM.
"""

import sys

if "/opt/trn_rl_repo" not in sys.path:
    sys.path.insert(0, "/opt/trn_rl_repo")

import math

import numpy as np

B, S, D, H = 2, 2048, 4096, 32
HD = D // H          # 128
HLOC = 4             # heads per core
NC = 8               # cores
TOK = B * S          # 4096
CH = TOK // 512      # 8 token chunks of 512
DKT = D // 128       # 32 contraction tiles
QC = S // 512        # 4 q-chunks per sequence
KT = S // 128        # 16 k-tiles per sequence
ISQRT = 1.0 / math.sqrt(HD)

_CACHE = {}


def _build(causal: bool, nrep: int = 1):
    import concourse.bacc as bacc
    import concourse.tile as tile
    from concourse import mybir

    F32 = mybir.dt.float32
    F32R = mybir.dt.float32r
    EXP = mybir.ActivationFunctionType.Exp

    nc = bacc.Bacc("TRN2", target_bir_lowering=False, debug=False, num_devices=NC)

    xt_d = nc.dram_tensor("xt", [DKT, 128, TOK], F32R, kind="ExternalInput")
    wq_d = nc.dram_tensor("wq", [128, DKT, 512], F32R, kind="ExternalInput")
    wk_d = nc.dram_tensor("wk", [128, DKT, 512], F32R, kind="ExternalInput")
    wv_d = nc.dram_tensor("wv", [128, DKT, 512], F32R, kind="ExternalInput")
    wo_d = nc.dram_tensor("wo", [128, HLOC, D], F32R, kind="ExternalInput")
    cs_d = nc.dram_tensor("cs", [128, S], F32, kind="ExternalInput")
    ss_d = nc.dram_tensor("ss", [128, S], F32, kind="ExternalInput")
    if causal:
        mk_d = nc.dram_tensor("maskd", [128, 4, 512], F32, kind="ExternalInput")
    else:
        mk_d = nc.dram_tensor("maskf", [KT, 128, S], F32, kind="ExternalInput")
    out_d = nc.dram_tensor("out", [TOK, D], F32, kind="ExternalOutput")

    # DRAM scratch for projected Q/K/V — one tensor per (head, batch) so
    # phase-2 loads depend only on the spills they actually read
    qdr = {(h, b): nc.dram_tensor(f"qdr{h}_{b}", [128, S], F32R)
           for h in range(HLOC) for b in range(B)}
    kdr = {(h, b): nc.dram_tensor(f"kdr{h}_{b}", [128, S], F32R)
           for h in range(HLOC) for b in range(B)}
    vdr = {b: nc.dram_tensor(f"vdr{b}", [S, 512], F32R) for b in range(B)}

    with tile.TileContext(nc) as tc:
        with (
            tc.tile_pool(name="consts", bufs=1) as consts,
        ):
            ones_sb = consts.tile([128, 1], F32R)
            nc.vector.memset(ones_sb.bitcast(F32), 1.0)
            mkd_sb = consts.tile([128, 4, 512], F32, name="mkd") if causal else None
            for _it in range(nrep):
                _emit_iter(nc, tc, _it, causal, ones_sb, mkd_sb, xt_d, wq_d,
                           wk_d, wv_d, wo_d, cs_d, ss_d, mk_d, out_d,
                           qdr, kdr, vdr, F32, F32R, EXP)

    nc.compile()
    return nc


def _emit_iter(nc, tc, it, causal, ones_sb, mkd_sb, xt_d, wq_d, wk_d, wv_d,
               wo_d, cs_d, ss_d, mk_d, out_d, qdr, kdr, vdr, F32, F32R, EXP):
    if True:
        if True:

            # ---------------- Phase 1b: V projection (first: cold start
            # overlaps the wv load on the empty ACT queue) ----------------
            with (
                tc.tile_pool(name=f"w2{it}", bufs=1) as w2,
                tc.tile_pool(name=f"xt2{it}", bufs=4) as xt2,
                tc.tile_pool(name=f"vcp{it}", bufs=4) as vcp,
                tc.tile_pool(name=f"ps2{it}", bufs=2, space="PSUM") as ps2,
            ):
                wv_sb = w2.tile([128, DKT, 512], F32R, tag="wv")
                for ch in range(CH):
                    b, s0 = ch // QC, (ch % QC) * 512
                    vps = [ps2.tile([128, 512], F32, name=f"vps{t}", tag=f"v{t}") for t in range(4)]
                    for dk in range(DKT):
                        if ch == 0:
                            we = nc.scalar if dk % 2 == 0 else nc.sync
                            we.dma_start(
                                out=wv_sb[:, dk, :], in_=wv_d.ap()[:, dk, :]
                            )
                        xt = xt2.tile(
                            [128, 512], F32R, name="xt",
                            tag="xt0" if dk < 4 else "xt", bufs=3 if dk < 4 else None,
                        )
                        nc.sync.dma_start(
                            out=xt, in_=xt_d.ap()[dk, :, ch * 512:(ch + 1) * 512]
                        )
                        for t in range(4):
                            nc.tensor.matmul(
                                vps[t], xt[:, t * 128:(t + 1) * 128], wv_sb[:, dk, :],
                                start=(dk == 0), stop=(dk == DKT - 1),
                            )
                    for t in range(4):
                        vc = vcp.tile([128, 512], F32R, tag="vc")
                        nc.vector.tensor_copy(vc, vps[t])
                        nc.gpsimd.dma_start(
                            out=vdr[b].ap()[s0 + t * 128:s0 + (t + 1) * 128, :], in_=vc
                        )

            # ---------------- Phase 1a: Q,K projections + RoPE ----------
            with (
                tc.tile_pool(name=f"w1{it}", bufs=1) as w1,
                tc.tile_pool(name=f"xt1{it}", bufs=4) as xt1,
                tc.tile_pool(name=f"rope{it}", bufs=2) as rope,
                tc.tile_pool(name=f"ps1{it}", bufs=1, space="PSUM") as ps1,
            ):
                wq_sb = w1.tile([128, DKT, 512], F32R, tag="wq")
                wk_sb = w1.tile([128, DKT, 512], F32R, tag="wk")

                for ch in range(CH):
                    b, s0 = ch // QC, (ch % QC) * 512
                    cs_sb = rope.tile([128, 512], F32, name="cs_c", tag="cs_c")
                    ss_sb = rope.tile([128, 512], F32, name="ss_c", tag="ss_c")
                    nc.scalar.dma_start(out=cs_sb, in_=cs_d.ap()[:, s0:s0 + 512])
                    nc.scalar.dma_start(out=ss_sb, in_=ss_d.ap()[:, s0:s0 + 512])
                    qps = [ps1.tile([128, 512], F32, name=f"qps{h}", tag=f"q{h}") for h in range(HLOC)]
                    kps = [ps1.tile([128, 512], F32, name=f"kps{h}", tag=f"k{h}") for h in range(HLOC)]
                    for dk in range(DKT):
                        if ch == 0:
                            we = nc.scalar if dk % 2 == 0 else nc.sync
                            wf = nc.sync if dk % 2 == 0 else nc.scalar
                            we.dma_start(
                                out=wq_sb[:, dk, :], in_=wq_d.ap()[:, dk, :]
                            )
                            wf.dma_start(
                                out=wk_sb[:, dk, :], in_=wk_d.ap()[:, dk, :]
                            )
                        xt = xt1.tile(
                            [128, 512], F32R, name="xt",
                            tag="xt0" if dk < 4 else "xt", bufs=3 if dk < 4 else None,
                        )
                        nc.sync.dma_start(
                            out=xt, in_=xt_d.ap()[dk, :, ch * 512:(ch + 1) * 512]
                        )
                        for h in range(HLOC):
                            nc.tensor.matmul(
                                qps[h], wq_sb[:, dk, h * 128:(h + 1) * 128], xt,
                                start=(dk == 0), stop=(dk == DKT - 1),
                            )
                        for h in range(HLOC):
                            nc.tensor.matmul(
                                kps[h], wk_sb[:, dk, h * 128:(h + 1) * 128], xt,
                                start=(dk == 0), stop=(dk == DKT - 1),
                            )
                    # epilogue pass 1: drain all PSUM banks first (frees the
                    # pool for the next chunk / phase 2), swaps issued eagerly
                    work = []
                    for h in range(HLOC):
                        for ps, dst in ((qps[h], qdr), (kps[h], kdr)):
                            pc = rope.tile([128, 512], F32, name="pc", tag="pc",
                                           bufs=4)
                            t1 = rope.tile([128, 512], F32, name="t1", tag="t1",
                                           bufs=8)
                            s1 = rope.tile([128, 512], F32, name="s1", tag="s1",
                                           bufs=2)
                            s1w = rope.tile([128, 512], F32, name="s1w", tag="s1w",
                                            bufs=8)
                            nc.vector.tensor_copy(pc, ps)
                            nc.vector.tensor_mul(t1, pc, cs_sb)
                            nc.vector.tensor_mul(s1, pc, ss_sb)
                            nc.scalar.dma_start(out=s1w[0:64, :], in_=s1[64:128, :])
                            nc.scalar.dma_start(out=s1w[64:128, :], in_=s1[0:64, :])
                            work.append((h, dst, t1, s1w))
                    # epilogue pass 2: combine + spill
                    for h, dst, t1, s1w in work:
                        rr = rope.tile([128, 512], F32R, name="rr", tag="rr", bufs=2)
                        nc.vector.tensor_add(rr, t1, s1w)
                        nc.scalar.dma_start(
                            out=dst[(h, b)].ap()[:, s0:s0 + 512], in_=rr
                        )

            # ---------------- Phases 2+3 per batch ----------------------
            if causal and it == 0:
                nc.scalar.dma_start(out=mkd_sb, in_=mk_d.ap())
            _p23(nc, tc, it, causal, ones_sb, mkd_sb if causal else mk_d,
                 wo_d, qdr, kdr, vdr, out_d, F32, F32R, EXP)


def _p23(nc, tc, it, causal, ones_sb, mk, wo_d, qdr, kdr, vdr, out_d, F32, F32R, EXP):
    hbs = [(b, h) for b in range(B) for h in range(HLOC)]

    with (
        tc.tile_pool(name=f"qkv{it}", bufs=2) as qkv,
        tc.tile_pool(name=f"hold{it}", bufs=1) as hold,
        tc.tile_pool(name=f"sm{it}", bufs=2) as sm,
        tc.tile_pool(name=f"ps3{it}", bufs=1, space="PSUM") as ps3,
        tc.tile_pool(name=f"ps4{it}", bufs=3, space="PSUM") as ps4,
    ):
        def load_hb(i):
            b, h = hbs[i]
            qT = qkv.tile([128, S], F32R, name=f"qT{it}_{i}", tag="qT")
            kT = qkv.tile([128, S], F32R, name=f"kT{it}_{i}", tag="kT")
            vT = qkv.tile([128, KT, 128], F32R, name=f"vT{it}_{i}", tag="vT")
            vsrc = vdr[b].ap()[:, h * 128:(h + 1) * 128].rearrange(
                "(n p) d -> p n d", p=128
            )
            for j in range(QC):
                sl = slice(j * 512, (j + 1) * 512)
                nc.sync.dma_start(out=qT[:, sl], in_=qdr[(h, b)].ap()[:, sl])
                nc.sync.dma_start(out=kT[:, sl], in_=kdr[(h, b)].ap()[:, sl])
                nc.sync.dma_start(
                    out=vT[:, j * 4:(j + 1) * 4, :],
                    in_=vsrc[:, j * 4:(j + 1) * 4, :],
                )
            return qT, kT, vT

        tiles = {0: load_hb(0)}
        wo_sb = hold.tile([128, HLOC, D], F32R, tag="wo")
        for h in range(HLOC):
            nc.scalar.dma_start(out=wo_sb[:, h, :], in_=wo_d.ap()[:, h, :])

        hoTs = {}
        for i, (b, h) in enumerate(hbs):
            if h == 0:
                hoTs[b] = hold.tile([128, HLOC, S], F32R, name=f"hoT{it}_{b}",
                                    tag=f"hoT{b}")
            hoT = hoTs[b]
            if i + 1 < len(hbs):
                tiles[i + 1] = load_hb(i + 1)
            qT, kT, vT = tiles.pop(i)
            # 512-wide q-chunks (N=512 keeps LDWEIGHTS fully hidden on HW;
            # measured faster than 256-wide despite coarser causal skipping)
            for qc in range(QC):
                qs = qc * 512
                nkt = (qc + 1) * 4 if causal else KT
                sums = ps3.tile([1, 512], F32, name="sums", tag="sums")
                hops = ps3.tile([128, 512], F32, name="hops", tag="hops")
                for kt in range(nkt):
                    st = ps4.tile([128, 512], F32, name="st", tag="st")
                    nc.tensor.matmul(
                        st, kT[:, kt * 128:(kt + 1) * 128],
                        qT[:, qs:qs + 512],
                        start=True, stop=True,
                    )
                    if causal:
                        if kt >= nkt - 4:
                            nc.vector.tensor_add(
                                st, st, mk[:, kt - (nkt - 4), :]
                            )
                    else:
                        mkt = sm.tile([128, 512], F32, name="mkt", tag="mkt")
                        nc.sync.dma_start(
                            out=mkt, in_=mk.ap()[kt, :, qs:qs + 512]
                        )
                        nc.vector.tensor_add(st, st, mkt)
                    ex = sm.tile([128, 512], F32R, name="ex", tag="ex", bufs=4)
                    nc.scalar.activation(ex, st, EXP, scale=ISQRT)
                    nc.tensor.matmul(
                        sums, ones_sb, ex, start=(kt == 0), stop=(kt == nkt - 1)
                    )
                    nc.tensor.matmul(
                        hops, vT[:, kt, :], ex, start=(kt == 0), stop=(kt == nkt - 1)
                    )
                recip = sm.tile([1, 512], F32, name="recip", tag="recip")
                nc.vector.reciprocal(recip, sums)
                bc = sm.tile([128, 512], F32, name="bc", tag="bc")
                nc.gpsimd.partition_broadcast(bc, recip)
                nc.vector.tensor_mul(hoT[:, h, qs:qs + 512], hops, bc)

        for b in range(B):
            _p3(nc, tc, it, b, hoTs[b], wo_sb, out_d, F32)


def _p3(nc, tc, it, b, hoT, wo_sb, out_d, F32):
    """Output projection for one batch: out[tok,:] = sum_h hoT_h' @ wo_h."""
    with (
        tc.tile_pool(name=f"oc{it}_{b}", bufs=3) as ocp,
        tc.tile_pool(name=f"ps5{it}_{b}", bufs=3, space="PSUM") as ps5,
    ):
        for t in range(S // 128):
            for oc in range(D // 512):
                ops = ps5.tile([128, 512], F32, name="ops", tag="ops")
                for h in range(HLOC):
                    nc.tensor.matmul(
                        ops, hoT[:, h, t * 128:(t + 1) * 128],
                        wo_sb[:, h, oc * 512:(oc + 1) * 512],
                        start=(h == 0), stop=(h == HLOC - 1),
                    )
                ot = ocp.tile([128, 512], F32, name="ot", tag="ot")
                nc.vector.tensor_copy(ot, ops)
                nc.scalar.dma_start(
                    out=out_d.ap()[
                        b * S + t * 128:b * S + (t + 1) * 128,
                        oc * 512:(oc + 1) * 512,
                    ],
                    in_=ot,
                )


def _get_nc(causal: bool):
    if causal not in _CACHE:
        _CACHE[causal] = _build(causal)
    return _CACHE[causal]


def _host_prep(x, wq, wk, wv, wo, freqs_cos, freqs_sin, mask):
    """Build per-core input maps."""
    x2 = np.ascontiguousarray(x.reshape(TOK, D).T)          # [D, TOK]
    xt = x2.reshape(DKT, 128, TOK)

    cs = np.concatenate([freqs_cos.T, freqs_cos.T], axis=0).astype(np.float32)
    ss = np.concatenate([freqs_sin.T, -freqs_sin.T], axis=0).astype(np.float32)

    m2 = np.asarray(mask, dtype=np.float32).reshape(S, S)
    # causal iff: zero on/below diagonal, <= -1e8 strictly above
    tril = np.tril(np.ones((S, S), dtype=bool))
    causal = bool(np.all(m2[tril] == 0.0) and np.all(m2[~tril] <= -1e8))
    if causal:
        mk = np.ascontiguousarray(
            m2[:512, :512].T.reshape(4, 128, 512).transpose(1, 0, 2)
        )
    else:
        mk = np.ascontiguousarray(m2.T.reshape(KT, 128, S))

    # per-head column permutation: evens then odds (RoPE rotate-half form)
    perm = np.concatenate([np.arange(0, HD, 2), np.arange(1, HD, 2)])

    in_maps = []
    for c in range(NC):
        cols = np.concatenate(
            [(4 * c + h) * HD + perm for h in range(HLOC)]
        )
        wq_c = np.ascontiguousarray(
            wq[:, cols].reshape(DKT, 128, 512).transpose(1, 0, 2)
        )
        wk_c = np.ascontiguousarray(
            wk[:, cols].reshape(DKT, 128, 512).transpose(1, 0, 2)
        )
        vcols = np.arange(4 * c * HD, 4 * (c + 1) * HD)
        wv_c = np.ascontiguousarray(
            wv[:, vcols].reshape(DKT, 128, 512).transpose(1, 0, 2)
        )
        wo_c = np.ascontiguousarray(
            wo[vcols, :].reshape(HLOC, 128, D).transpose(1, 0, 2)
        )
        m = {
            "xt": xt, "wq": wq_c, "wk": wk_c, "wv": wv_c, "wo": wo_c,
            "cs": cs, "ss": ss,
        }
        m["maskd" if causal else "maskf"] = mk
        in_maps.append(m)
    return in_maps, causal


def kernel(x, wq, wk, wv, wo, freqs_cos, freqs_sin, mask, **_unused):
    from concourse.bass_utils import run_bass_kernel_spmd

    x = np.asarray(x, dtype=np.float32)
    wq = np.asarray(wq, dtype=np.float32)
    wk = np.asarray(wk, dtype=np.float32)
    wv = np.asarray(wv, dtype=np.float32)
    wo = np.asarray(wo, dtype=np.float32)
    freqs_cos = np.asarray(freqs_cos, dtype=np.float32)
    freqs_sin = np.asarray(freqs_sin, dtype=np.float32)

    in_maps, causal = _host_prep(x, wq, wk, wv, wo, freqs_cos, freqs_sin, mask)
    nc = _get_nc(causal)
    res = run_bass_kernel_spmd(nc, in_maps, list(range(NC)))
    out = res.results[0]["out"]
    for c in range(1, NC):
        out = out + res.results[c]["out"]
    return out.reshape(B, S, D).astype(np.float32)



# revision 8
# speedup vs baseline: 1.0250x; 1.0250x over previous
"""Trainium2 Bass kernel for multi-head causal attention with RoPE.

Model (per reference):
  B=2, S=2048, D=4096, H=32 heads, HD=128.
  out = softmax(rope(x@wq) @ rope(x@wk)^T / sqrt(HD) + mask) @ (x@wv) @ wo

Sharding: tensor-parallel over heads. Core c in 0..7 owns heads 4c..4c+3:
wq/wk/wv column-sharded, wo row-sharded; each core produces a full-shape
partial output and the host sums the 8 partials (the all-reduce).

Per-core pipeline (matmuls in fp32r — ~tf32 accuracy, 4x faster than fp32
at N>=256):
  Phase 1b: V projection in natural [tok, d] layout, 512-token chunks with
            double-buffered PSUM, spilled to DRAM.  wq is prefetched on
            the gpsimd queue during this phase so phase 1a starts hot.
  Phase 1a: Q,K projections emitted in transposed [HD, tok] layout
            (lhsT = weight tile, rhs = xT tile), RoPE applied in
            rotate-half form (host permutes wq/wk columns evens-then-odds
            per head so the pair-swap becomes a partition-block swap done
            with two SBUF->SBUF DMAs), spilled to DRAM scratch.  The PSUM
            drain copies alternate DVE/ACT so the next chunk's matmuls
            start sooner.
  Phase 2:  per (head,batch): S^T[k,q] = K^T-tile' @ Q^T chunk computed
            512 q-columns at a time; causal mask added only on diagonal
            128x512 blocks with the fully-masked column prefix sliced off
            (widths 512/384/256/256 — the 256 floor keeps fp32r at 1
            cycle/row), k-tiles above the diagonal skipped; exp on ScalarE
            (scores are bounded, softmax is shift-invariant so skipping
            the max subtraction matches the reference up to rounding);
            column sums via ones-vector matmul; PV accumulated in PSUM as
            ho^T[d,q]; normalization applied after PV via a gpsimd
            partition-broadcast reciprocal.  qT/kT reloads ride the gpsimd
            queue (vT on sync) so they start the moment the spills land.
  Phase 3:  out_partial[tok,:] += ho^T_h' @ wo_rows_h accumulated over
            the 4 local heads in PSUM, 6 banks in flight, PSUM drained by
            copies alternating DVE/ACT and stores alternating scalar/sync.
"""

import sys

if "/opt/trn_rl_repo" not in sys.path:
    sys.path.insert(0, "/opt/trn_rl_repo")

import math

import numpy as np

B, S, D, H = 2, 2048, 4096, 32
HD = D // H          # 128
HLOC = 4             # heads per core
NC = 8               # cores
TOK = B * S          # 4096
CH = TOK // 512      # 8 token chunks of 512
DKT = D // 128       # 32 contraction tiles
QC = S // 512        # 4 q-chunks per sequence
KT = S // 128        # 16 k-tiles per sequence
ISQRT = 1.0 / math.sqrt(HD)

_CACHE = {}


def _build(causal: bool):
    import concourse.bacc as bacc
    import concourse.tile as tile
    from concourse import mybir

    F32 = mybir.dt.float32
    F32R = mybir.dt.float32r
    EXP = mybir.ActivationFunctionType.Exp
    CPY = mybir.ActivationFunctionType.Copy

    nc = bacc.Bacc("TRN2", target_bir_lowering=False, debug=False, num_devices=NC)

    xt_d = nc.dram_tensor("xt", [DKT, 128, TOK], F32R, kind="ExternalInput")
    wq_d = nc.dram_tensor("wq", [128, DKT, 512], F32R, kind="ExternalInput")
    wk_d = nc.dram_tensor("wk", [128, DKT, 512], F32R, kind="ExternalInput")
    wv_d = nc.dram_tensor("wv", [128, DKT, 512], F32R, kind="ExternalInput")
    wo_d = nc.dram_tensor("wo", [128, HLOC, D], F32R, kind="ExternalInput")
    cs_d = nc.dram_tensor("cs", [128, S], F32, kind="ExternalInput")
    ss_d = nc.dram_tensor("ss", [128, S], F32, kind="ExternalInput")
    if causal:
        mk_d = nc.dram_tensor("maskd", [128, 4, 512], F32, kind="ExternalInput")
    else:
        mk_d = nc.dram_tensor("maskf", [KT, 128, S], F32, kind="ExternalInput")
    out_d = nc.dram_tensor("out", [TOK, D], F32, kind="ExternalOutput")

    # DRAM scratch for projected Q/K/V — one tensor per (head, batch) so
    # phase-2 loads depend only on the spills they actually read
    qdr = {(h, b): nc.dram_tensor(f"qdr{h}_{b}", [128, S], F32R)
           for h in range(HLOC) for b in range(B)}
    kdr = {(h, b): nc.dram_tensor(f"kdr{h}_{b}", [128, S], F32R)
           for h in range(HLOC) for b in range(B)}
    vdr = {b: nc.dram_tensor(f"vdr{b}", [S, 512], F32R) for b in range(B)}

    with tile.TileContext(nc) as tc:
        with tc.tile_pool(name="consts", bufs=1) as consts:
            ones_sb = consts.tile([128, 1], F32R)
            nc.vector.memset(ones_sb.bitcast(F32), 1.0)
            mkd_sb = consts.tile([128, 4, 512], F32, name="mkd") if causal else None

            with tc.tile_pool(name="w1q", bufs=1) as w1q:
                wq_sb = w1q.tile([128, DKT, 512], F32R, tag="wq")
                _p1b(nc, tc, xt_d, wv_d, vdr, wq_d, wq_sb, F32, F32R)
                _p1a(nc, tc, xt_d, wk_d, wq_sb, cs_d, ss_d,
                     qdr, kdr, F32, F32R, CPY)

            if causal:
                nc.scalar.dma_start(out=mkd_sb, in_=mk_d.ap())
            _p23(nc, tc, causal, ones_sb, mkd_sb if causal else mk_d,
                 wo_d, qdr, kdr, vdr, out_d, F32, F32R, EXP, CPY)

    nc.compile()
    return nc


def _p1b(nc, tc, xt_d, wv_d, vdr, wq_d, wq_sb, F32, F32R):
    """V projection: out[tok, 512] in 512-token chunks, PSUM x2 buffered.
    Streams the wq loads for phase 1a through the gpsimd queue."""
    with (
        tc.tile_pool(name="w2", bufs=1) as w2,
        tc.tile_pool(name="xt2", bufs=4) as xt2,
        tc.tile_pool(name="vcp", bufs=4) as vcp,
        tc.tile_pool(name="ps2", bufs=2, space="PSUM") as ps2,
    ):
        wv_sb = w2.tile([128, DKT, 512], F32R, tag="wv")
        for ch in range(CH):
            b, s0 = ch // QC, (ch % QC) * 512
            for dk in range(4 * ch, 4 * ch + 4):
                nc.gpsimd.dma_start(out=wq_sb[:, dk, :], in_=wq_d.ap()[:, dk, :])
            vps = [ps2.tile([128, 512], F32, name=f"vps{t}", tag=f"v{t}")
                   for t in range(4)]
            for dk in range(DKT):
                if ch == 0:
                    nc.scalar.dma_start(
                        out=wv_sb[:, dk, :], in_=wv_d.ap()[:, dk, :]
                    )
                xt = xt2.tile(
                    [128, 512], F32R, name="xt",
                    tag="xt0" if dk < 4 else "xt", bufs=3 if dk < 4 else None,
                )
                nc.sync.dma_start(
                    out=xt, in_=xt_d.ap()[dk, :, ch * 512:(ch + 1) * 512]
                )
                for t in range(4):
                    nc.tensor.matmul(
                        vps[t], xt[:, t * 128:(t + 1) * 128], wv_sb[:, dk, :],
                        start=(dk == 0), stop=(dk == DKT - 1),
                    )
            for t in range(4):
                vc = vcp.tile([128, 512], F32R, tag="vc")
                nc.vector.tensor_copy(vc, vps[t])
                nc.gpsimd.dma_start(
                    out=vdr[b].ap()[s0 + t * 128:s0 + (t + 1) * 128, :], in_=vc
                )


def _p1a(nc, tc, xt_d, wk_d, wq_sb, cs_d, ss_d, qdr, kdr,
         F32, F32R, CPY):
    """Q,K projections + RoPE, 512-token chunks."""
    with (
        tc.tile_pool(name="w1k", bufs=1) as w1k,
        tc.tile_pool(name="xt1", bufs=4) as xt1,
        tc.tile_pool(name="rope", bufs=2) as rope,
        tc.tile_pool(name="ps1", bufs=1, space="PSUM") as ps1,
    ):
        wk_sb = w1k.tile([128, DKT, 512], F32R, tag="wk")
        for ch in range(CH):
            b, s0 = ch // QC, (ch % QC) * 512
            cs_sb = rope.tile([128, 512], F32, name="cs_c", tag="cs_c")
            ss_sb = rope.tile([128, 512], F32, name="ss_c", tag="ss_c")
            nc.scalar.dma_start(out=cs_sb, in_=cs_d.ap()[:, s0:s0 + 512])
            nc.scalar.dma_start(out=ss_sb, in_=ss_d.ap()[:, s0:s0 + 512])
            qps = [ps1.tile([128, 512], F32, name=f"qps{h}", tag=f"q{h}")
                   for h in range(HLOC)]
            kps = [ps1.tile([128, 512], F32, name=f"kps{h}", tag=f"k{h}")
                   for h in range(HLOC)]
            for dk in range(DKT):
                if ch == 0:
                    we = nc.scalar if dk % 2 == 0 else nc.sync
                    we.dma_start(out=wk_sb[:, dk, :], in_=wk_d.ap()[:, dk, :])
                xt = xt1.tile(
                    [128, 512], F32R, name="xt",
                    tag="xt0" if dk < 4 else "xt", bufs=3 if dk < 4 else None,
                )
                nc.sync.dma_start(
                    out=xt, in_=xt_d.ap()[dk, :, ch * 512:(ch + 1) * 512]
                )
                for h in range(HLOC):
                    nc.tensor.matmul(
                        qps[h], wq_sb[:, dk, h * 128:(h + 1) * 128], xt,
                        start=(dk == 0), stop=(dk == DKT - 1),
                    )
                for h in range(HLOC):
                    nc.tensor.matmul(
                        kps[h], wk_sb[:, dk, h * 128:(h + 1) * 128], xt,
                        start=(dk == 0), stop=(dk == DKT - 1),
                    )
            # epilogue pass 1: drain all PSUM banks first (frees the pool for
            # the next chunk), copies alternate DVE/ACT, swaps issued eagerly
            work = []
            for i, (h, ps, dst) in enumerate(
                (h, ps, dst) for h in range(HLOC)
                for ps, dst in ((qps[h], qdr), (kps[h], kdr))
            ):
                pc = rope.tile([128, 512], F32, name="pc", tag="pc", bufs=4)
                t1 = rope.tile([128, 512], F32, name="t1", tag="t1", bufs=8)
                s1 = rope.tile([128, 512], F32, name="s1", tag="s1", bufs=2)
                s1w = rope.tile([128, 512], F32, name="s1w", tag="s1w", bufs=8)
                if i % 2 == 0:
                    nc.vector.tensor_copy(pc, ps)
                else:
                    nc.scalar.activation(pc, ps, CPY)
                nc.vector.tensor_mul(t1, pc, cs_sb)
                nc.vector.tensor_mul(s1, pc, ss_sb)
                nc.scalar.dma_start(out=s1w[0:64, :], in_=s1[64:128, :])
                nc.scalar.dma_start(out=s1w[64:128, :], in_=s1[0:64, :])
                work.append((h, dst, t1, s1w))
            # epilogue pass 2: combine + spill
            for h, dst, t1, s1w in work:
                rr = rope.tile([128, 512], F32R, name="rr", tag="rr", bufs=2)
                nc.vector.tensor_add(rr, t1, s1w)
                nc.scalar.dma_start(out=dst[(h, b)].ap()[:, s0:s0 + 512], in_=rr)


def _p23(nc, tc, causal, ones_sb, mk, wo_d, qdr, kdr, vdr, out_d,
         F32, F32R, EXP, CPY):
    hbs = [(b, h) for b in range(B) for h in range(HLOC)]

    with tc.tile_pool(name="hold", bufs=1) as hold:
        wo_sb = hold.tile([128, HLOC, D], F32R, tag="wo")
        for h in range(HLOC):
            nc.scalar.dma_start(out=wo_sb[:, h, :], in_=wo_d.ap()[:, h, :])
        hoTs = {b: hold.tile([128, HLOC, S], F32R, name=f"hoT{b}", tag=f"hoT{b}")
                for b in range(B)}

        with (
            tc.tile_pool(name="qkv", bufs=2) as qkv,
            tc.tile_pool(name="sm", bufs=2) as sm,
            tc.tile_pool(name="ps3", bufs=2, space="PSUM") as ps3,
            tc.tile_pool(name="ps4", bufs=3, space="PSUM") as ps4,
        ):
            def load_hb(i):
                b, h = hbs[i]
                qT = qkv.tile([128, S], F32R, name=f"qT{i}", tag="qT")
                kT = qkv.tile([128, S], F32R, name=f"kT{i}", tag="kT")
                vT = qkv.tile([128, KT, 128], F32R, name=f"vT{i}", tag="vT")
                vsrc = vdr[b].ap()[:, h * 128:(h + 1) * 128].rearrange(
                    "(n p) d -> p n d", p=128
                )
                for j in range(QC):
                    sl = slice(j * 512, (j + 1) * 512)
                    nc.gpsimd.dma_start(out=qT[:, sl], in_=qdr[(h, b)].ap()[:, sl])
                    nc.gpsimd.dma_start(out=kT[:, sl], in_=kdr[(h, b)].ap()[:, sl])
                    nc.sync.dma_start(
                        out=vT[:, j * 4:(j + 1) * 4, :],
                        in_=vsrc[:, j * 4:(j + 1) * 4, :],
                    )
                return qT, kT, vT

            tiles = {0: load_hb(0)}
            for i, (b, h) in enumerate(hbs):
                hoT = hoTs[b]
                if i + 1 < len(hbs):
                    tiles[i + 1] = load_hb(i + 1)
                qT, kT, vT = tiles.pop(i)
                for qc in range(QC):
                    qs = qc * 512
                    nkt = (qc + 1) * 4 if causal else KT
                    sums = ps3.tile([1, 512], F32, name="sums", tag="sums")
                    hops = ps3.tile([128, 512], F32, name="hops", tag="hops")
                    for kt in range(nkt):
                        d = kt - (nkt - 4) if causal else -1
                        qo = 0 if d < 0 else min(d * 128, 256)
                        st = ps4.tile([128, 512], F32, name="st", tag="st")
                        nc.tensor.matmul(
                            st[:, qo:], kT[:, kt * 128:(kt + 1) * 128],
                            qT[:, qs + qo:qs + 512],
                            start=True, stop=True,
                        )
                        if causal:
                            if d >= 0:
                                nc.vector.tensor_add(
                                    st[:, qo:], st[:, qo:], mk[:, d, qo:]
                                )
                        else:
                            mkt = sm.tile([128, 512], F32, name="mkt", tag="mkt")
                            nc.sync.dma_start(
                                out=mkt, in_=mk.ap()[kt, :, qs:qs + 512]
                            )
                            nc.vector.tensor_add(st, st, mkt)
                        ex = sm.tile([128, 512], F32R, name="ex", tag="ex", bufs=5)
                        nc.scalar.activation(ex[:, qo:], st[:, qo:], EXP,
                                             scale=ISQRT)
                        nc.tensor.matmul(
                            sums[:, qo:], ones_sb, ex[:, qo:],
                            start=(kt == 0), stop=(kt == nkt - 1),
                        )
                        nc.tensor.matmul(
                            hops[:, qo:], vT[:, kt, :], ex[:, qo:],
                            start=(kt == 0), stop=(kt == nkt - 1),
                        )
                    recip = sm.tile([1, 512], F32, name="recip", tag="recip")
                    nc.vector.reciprocal(recip, sums)
                    bc = sm.tile([128, 512], F32, name="bc", tag="bc")
                    nc.gpsimd.partition_broadcast(bc, recip)
                    nc.vector.tensor_mul(hoT[:, h, qs:qs + 512], hops, bc)

        for b in range(B):
            _p3(nc, tc, b, hoTs[b], wo_sb, out_d, F32, CPY)


def _p3(nc, tc, b, hoT, wo_sb, out_d, F32, CPY):
    """Output projection for one batch: out[tok,:] = sum_h hoT_h' @ wo_h."""
    with (
        tc.tile_pool(name=f"oc{b}", bufs=4) as ocp,
        tc.tile_pool(name=f"ps5{b}", bufs=6, space="PSUM") as ps5,
    ):
        for t in range(S // 128):
            for oc in range(D // 512):
                ops = ps5.tile([128, 512], F32, name="ops", tag="ops")
                for h in range(HLOC):
                    nc.tensor.matmul(
                        ops, hoT[:, h, t * 128:(t + 1) * 128],
                        wo_sb[:, h, oc * 512:(oc + 1) * 512],
                        start=(h == 0), stop=(h == HLOC - 1),
                    )
                ot = ocp.tile([128, 512], F32, name="ot", tag="ot")
                if oc % 2 == 0:
                    nc.vector.tensor_copy(ot, ops)
                else:
                    nc.scalar.activation(ot, ops, CPY)
                oe = nc.scalar if oc % 2 == 0 else nc.sync
                oe.dma_start(
                    out=out_d.ap()[
                        b * S + t * 128:b * S + (t + 1) * 128,
                        oc * 512:(oc + 1) * 512,
                    ],
                    in_=ot,
                )


def _get_nc(causal: bool):
    if causal not in _CACHE:
        _CACHE[causal] = _build(causal)
    return _CACHE[causal]


def _host_prep(x, wq, wk, wv, wo, freqs_cos, freqs_sin, mask):
    """Build per-core input maps."""
    x2 = np.ascontiguousarray(x.reshape(TOK, D).T)          # [D, TOK]
    xt = x2.reshape(DKT, 128, TOK)

    cs = np.concatenate([freqs_cos.T, freqs_cos.T], axis=0).astype(np.float32)
    ss = np.concatenate([freqs_sin.T, -freqs_sin.T], axis=0).astype(np.float32)

    m2 = np.asarray(mask, dtype=np.float32).reshape(S, S)
    # causal iff: zero on/below diagonal, <= -1e8 strictly above
    tril = np.tril(np.ones((S, S), dtype=bool))
    causal = bool(np.all(m2[tril] == 0.0) and np.all(m2[~tril] <= -1e8))
    if causal:
        mk = np.ascontiguousarray(
            m2[:512, :512].T.reshape(4, 128, 512).transpose(1, 0, 2)
        )
    else:
        mk = np.ascontiguousarray(m2.T.reshape(KT, 128, S))

    # per-head column permutation: evens then odds (RoPE rotate-half form)
    perm = np.concatenate([np.arange(0, HD, 2), np.arange(1, HD, 2)])

    in_maps = []
    for c in range(NC):
        cols = np.concatenate(
            [(4 * c + h) * HD + perm for h in range(HLOC)]
        )
        wq_c = np.ascontiguousarray(
            wq[:, cols].reshape(DKT, 128, 512).transpose(1, 0, 2)
        )
        wk_c = np.ascontiguousarray(
            wk[:, cols].reshape(DKT, 128, 512).transpose(1, 0, 2)
        )
        vcols = np.arange(4 * c * HD, 4 * (c + 1) * HD)
        wv_c = np.ascontiguousarray(
            wv[:, vcols].reshape(DKT, 128, 512).transpose(1, 0, 2)
        )
        wo_c = np.ascontiguousarray(
            wo[vcols, :].reshape(HLOC, 128, D).transpose(1, 0, 2)
        )
        m = {
            "xt": xt, "wq": wq_c, "wk": wk_c, "wv": wv_c, "wo": wo_c,
            "cs": cs, "ss": ss,
        }
        m["maskd" if causal else "maskf"] = mk
        in_maps.append(m)
    return in_maps, causal


def kernel(x, wq, wk, wv, wo, freqs_cos, freqs_sin, mask, **_unused):
    from concourse.bass_utils import run_bass_kernel_spmd

    x = np.asarray(x, dtype=np.float32)
    wq = np.asarray(wq, dtype=np.float32)
    wk = np.asarray(wk, dtype=np.float32)
    wv = np.asarray(wv, dtype=np.float32)
    wo = np.asarray(wo, dtype=np.float32)
    freqs_cos = np.asarray(freqs_cos, dtype=np.float32)
    freqs_sin = np.asarray(freqs_sin, dtype=np.float32)

    in_maps, causal = _host_prep(x, wq, wk, wv, wo, freqs_cos, freqs_sin, mask)
    nc = _get_nc(causal)
    res = run_bass_kernel_spmd(nc, in_maps, list(range(NC)))
    out = res.results[0]["out"]
    for c in range(1, NC):
        out = out + res.results[c]["out"]
    return out.reshape(B, S, D).astype(np.float32)
